# revision 1
# baseline (speedup 1.0000x reference)
"""Grader entry point: kernel(**inputs) -> full output.

Dev version: imports lstm_bass (will be inlined for submission).
Shards N_words across 8 cores, runs the Bass kernel, gathers outputs.
"""
import numpy as np

N_WORDS, N_CORES = 32768, 8
N_CORE = N_WORDS // N_CORES

LAST_EXEC_NS = None
_CACHE = {}


def kernel(char_indices, char_lengths, word_emb, E_char, W_ih, W_hh,
           b_ih, b_hh, W_lin, b_lin):
    global LAST_EXEC_NS
    import lstm_bass
    from concourse.bass_utils import run_bass_kernel_spmd

    if "nc" not in _CACHE:
        _CACHE["nc"] = lstm_bass.build(n_core=N_CORE, num_devices=N_CORES)
    nc = _CACHE["nc"]

    char_indices = np.asarray(char_indices)
    char_lengths = np.asarray(char_lengths)
    word_emb = np.asarray(word_emb, dtype=np.float32)

    in_maps = []
    for cid in range(N_CORES):
        s = slice(cid * N_CORE, (cid + 1) * N_CORE)
        in_maps.append(lstm_bass.make_in_map(
            char_indices[s], char_lengths[s], word_emb[s],
            np.asarray(E_char), np.asarray(W_ih), np.asarray(W_hh),
            np.asarray(b_ih), np.asarray(b_hh),
            np.asarray(W_lin), np.asarray(b_lin)))

    import os
    trace = bool(int(os.environ.get("KBENCH_TRACE", "0")))
    res = run_bass_kernel_spmd(nc, in_maps, core_ids=list(range(N_CORES)),
                               trace=trace)
    LAST_EXEC_NS = res.exec_time_ns

    out = np.empty((N_WORDS, 256), dtype=np.float32)
    for cid in range(N_CORES):
        out[cid * N_CORE:(cid + 1) * N_CORE] = res.results[cid]["outT"].T
    return out
